# revision 14
# baseline (speedup 1.0000x reference)
"""Trainium2 Bass kernel for C51 categorical projection (histogram binning).

Algorithm (per row, all f32):
  alpha = (reward + 0.1)/0.4 = 2.5 r + 0.25;  m = floor(alpha);  f = alpha - m
  not_done=0 rows are rewritten as a point mass sum(p)=1 at atom 25 (value 0),
  which is exact because (r+0.1)/0.4 + 0.99*25 = (r+10)/0.4.
  Atom j lands at position m + j + x_j with x_j = f - 0.01 j, |x_j| < 1, so it
  contributes to "taps" tau in {j-1, j, j+1} relative to m:
      q[tau] = p[tau](1-|x_tau|) + p[tau-1] relu(x_{tau-1}) + p[tau+1] relu(-x_{tau+1})
  Each row's 52 taps are scattered to absolute bins (m + tau) with
  nc.gpsimd.local_scatter (per-partition indices; f32 moved exactly as int16
  pairs), then bins <=0 and >=50 are fold-reduced into output bins 0 and 50.

Sharding: pure data parallel, batch split across 8 cores (65536 rows each).
Rows map to (partition p, column g) via row = p*512 + g; superblocks of
SUP=24 row-groups (3072 rows) share one set of wide DVE ops and issue two
local_scatter calls (scatter scratch caps one call at 12 segments).
"""
from contextlib import ExitStack

import numpy as np

import concourse.bacc as bacc
import concourse.bass as bass
import concourse.tile as tile
import concourse.dve_ops as dve_ops
from concourse import mybir
from concourse.bass_utils import run_bass_kernel_spmd
from concourse.dve_spec import Spec, Src0, Src1, One, Zero, lower, maxx
from concourse.dve_uop import DveOpSpec

# problem constants (hardcoded per harness contract)
BS = 524288
A = 51
N_CORES = 8
ROWS = BS // N_CORES            # 65536 rows per core
P = 128                         # SBUF partitions
G = ROWS // P                   # 512 row-groups per partition
NT = 52                         # taps tau = 0..51
PW = 54                         # padded probs row: [0, p0..p50, 0, 0]
PAD_LO = 14
W = PAD_LO + NT + 14            # 80 f32 bins per segment in scatter dst
T_SC = 12                       # segments per scatter call (scratch cap)
SUP = 2 * T_SC                  # segments per superblock

F32 = mybir.dt.float32
I16 = mybir.dt.int16
I32 = mybir.dt.int32
AX = mybir.AxisListType
OP = mybir.AluOpType


def _register_op(name: str, body, reference):
    """Register a custom DVE op with a runtime-computed uops sha."""
    for op in dve_ops.OPS:
        if op.name == name:
            return op
    spec = Spec(body=body, reference=reference)
    row = dve_ops._CUSTOM_DVE_ROW_BASE + len(dve_ops.OPS)
    shas = {}
    for ver in ("v3", "v4"):
        tmp = DveOpSpec(name=name, opcode=row, uops=lower(spec, ver=ver),
                        rd1_en=True)
        shas[ver] = tmp.sha(ver)
    op = dve_ops.DveOp(name, spec, subdim=False, uops_sha=shas)
    dve_ops.OPS.append(op)
    dve_ops.CUSTOM_DVE_SPECS[op.name] = spec
    dve_ops._SUB_OPCODE_FOR_NAME[op.name] = row
    return op


def _ref2(fn):
    def _r(in0, in1, s0, s1, imm2):
        a = in0.astype(np.float32)
        b = np.asarray(in1, dtype=np.float32).reshape(a.shape)
        return fn(a, b).astype(np.float32)
    return _r


TRI_MUL = _register_op(
    "TRI_MUL_ANT", Src0 * (One - maxx(Src1, Zero - Src1)),
    _ref2(lambda a, b: a * (1.0 - np.abs(b))))
G_MUL = _register_op(
    "G_MUL_ANT", Src0 * maxx(Src1, Zero),
    _ref2(lambda a, b: a * np.maximum(b, 0.0)))
H_MUL = _register_op(
    "H_MUL_ANT", Src0 * maxx(Zero - Src1, Zero),
    _ref2(lambda a, b: a * np.maximum(-b, 0.0)))


def _block_sizes(g_total: int) -> list[int]:
    sizes = []
    g = 0
    while g < g_total:
        t = min(SUP, g_total - g)
        sizes.append(t)
        g += t
    return sizes


def host_constants() -> dict[str, np.ndarray]:
    # neg_tau[g, j] = -0.01 * j for j = 0..51, tiled per segment
    neg_tau = (-0.01 * np.arange(NT, dtype=np.float64)).astype(np.float32)
    neg_tau_full = np.tile(neg_tau, SUP)[None, :]
    # cidx[g, c] = 2*W*(g % T_SC) + 2*PAD_LO + c  (int16), per-scatter-call window
    c = np.arange(2 * NT, dtype=np.int64)
    g = np.arange(SUP, dtype=np.int64) % T_SC
    cidx = (2 * W * g[:, None] + 2 * PAD_LO + c[None, :]).astype(np.int16)
    return {"neg_tau": neg_tau_full, "cidx": cidx.reshape(1, -1)}


def build_kernel(ctx: ExitStack, tc: tile.TileContext, outs, ins,
                 g_total: int = G, repeat: int = 1, ablate: set | None = None):
    ablate = ablate or set()
    nc = tc.nc
    reward_d, probs_d, nd_d, neg_tau_d, cidx_d = ins
    out_d = outs[0]

    r_v = reward_d.rearrange("(p g) o -> p (g o)", p=P)      # [128, G]
    n_v = nd_d.rearrange("(p g) o -> p (g o)", p=P)          # [128, G]
    p_v = probs_d.rearrange("(p g) a -> p g a", p=P)         # [128, G, 51]
    o_v = out_d.rearrange("(p g) a -> p g a", p=P)           # [128, G, 51]

    const = ctx.enter_context(tc.tile_pool(name="const", bufs=1))
    pre = ctx.enter_context(tc.tile_pool(name="pre", bufs=1))
    pool = ctx.enter_context(tc.tile_pool(name="blk", bufs=3))

    neg_tau = const.tile([P, SUP * NT], F32, name="neg_tau")
    nc.sync.dma_start(neg_tau[:], neg_tau_d[:].partition_broadcast(P))
    cidx = const.tile([P, SUP * 2 * NT], I16, name="cidx")
    nc.sync.dma_start(cidx[:], cidx_d[:].partition_broadcast(P))

    if repeat > 1:
        loop_cm = tc.For_i(0, repeat, 1)
        loop_cm.__enter__()

    # ---- prepass: per-row scalars [128, G] ----
    rt = pre.tile([P, g_total], F32, name="rt")
    nc.sync.dma_start(rt[:], r_v[:])
    ndt = pre.tile([P, g_total], F32, name="ndt")
    nc.sync.dma_start(ndt[:], n_v[:])

    alpha = pre.tile([P, g_total], F32, name="alpha")
    nc.vector.tensor_scalar(alpha[:], rt[:], 2.5, 0.25, OP.mult, OP.add)
    m0i = pre.tile([P, g_total], I32, name="m0i")
    nc.vector.tensor_copy(m0i[:], alpha[:])                  # round to nearest
    m0f = pre.tile([P, g_total], F32, name="m0f")
    nc.vector.tensor_copy(m0f[:], m0i[:])
    lt = pre.tile([P, g_total], F32, name="lt")
    nc.vector.tensor_tensor(lt[:], alpha[:], m0f[:], OP.is_lt)   # alpha < m0 -> 1.0
    mf = pre.tile([P, g_total], F32, name="mf")
    nc.vector.tensor_tensor(mf[:], m0f[:], lt[:], OP.subtract)   # m = floor(alpha)
    ft = pre.tile([P, g_total], F32, name="ft")
    nc.vector.tensor_tensor(ft[:], alpha[:], mf[:], OP.subtract)  # f in [0,1)
    m2f = pre.tile([P, g_total], F32, name="m2f")
    # clamp m to the representable shift range, then double (byte-pair index)
    nc.vector.tensor_scalar(m2f[:], mf[:], float(-PAD_LO), float(W - PAD_LO - NT),
                            OP.max, OP.min)
    nc.vector.tensor_scalar_mul(m2f[:], m2f[:], 2.0)
    m2 = pre.tile([P, g_total], I16, name="m2")
    nc.vector.tensor_copy(m2[:], m2f[:])
    ndc = pre.tile([P, g_total], F32, name="ndc")
    nc.vector.tensor_scalar(ndc[:], ndt[:], -1.0, 1.0, OP.mult, OP.add)  # 1 - nd

    # ---- per-superblock loop ----
    g0 = 0
    for t in _block_sizes(g_total):
        gs = slice(g0, g0 + t)

        ppad = pool.tile([P, SUP, PW], F32, tag="ppad", name="ppad")[:, :t]
        nc.sync.dma_start(ppad[:, :, 1:52], p_v[:, gs])
        nc.scalar.memzero(ppad[:, :, 0:1])
        nc.scalar.memzero(ppad[:, :, 52:54])

        # p~ = nd * p (in place); then p~[atom 25] += (1 - nd)  (sum(p) == 1)
        nd_b = ndt[:, gs].unsqueeze(2).broadcast_to((P, t, 51))
        nc.vector.tensor_tensor(ppad[:, :, 1:52], ppad[:, :, 1:52], nd_b, OP.mult)
        nc.vector.tensor_tensor(ppad[:, :, 26:27], ppad[:, :, 26:27],
                                ndc[:, gs].unsqueeze(2), OP.add)

        # x[j] = f - 0.01*j over j = 0..51
        x_t = pool.tile([P, SUP, NT], F32, tag="x", name="x")[:, :t]
        f_b = ft[:, gs].unsqueeze(2).broadcast_to((P, t, NT))
        nc.vector.tensor_tensor(x_t[:], neg_tau[:, : t * NT].rearrange(
            "p (g n) -> p g n", n=NT), f_b, OP.add)

        pm = ppad[:, :, 1:53]                                # p~[j], j = 0..51
        q_t = pool.tile([P, SUP, NT], F32, tag="q", name="q")[:, :t]
        if "q" in ablate:
            nc.vector.tensor_copy(q_t[:], pm)
        else:
            nc.vector._custom_dve(TRI_MUL, out=q_t[:], in0=pm, in1=x_t[:])
            gj = pool.tile([P, SUP, NT], F32, tag="gj", name="gj")[:, :t]
            nc.vector._custom_dve(G_MUL, out=gj[:], in0=pm, in1=x_t[:])
            hj = pool.tile([P, SUP, NT], F32, tag="hj", name="hj")[:, :t]
            nc.vector._custom_dve(H_MUL, out=hj[:], in0=pm, in1=x_t[:])
            nc.vector.tensor_tensor(q_t[:, :, 1:52], q_t[:, :, 1:52],
                                    gj[:, :, 0:51], OP.add)
            nc.vector.tensor_tensor(q_t[:, :, 0:51], q_t[:, :, 0:51],
                                    hj[:, :, 1:52], OP.add)

        # indices: idx[g, c] = 2*W*(g % T_SC) + 2*PAD_LO + c + 2*m  (int16)
        idx = pool.tile([P, SUP * 2 * NT], I16, tag="idx", name="idx")[:, : t * 2 * NT]
        m2_b = m2[:, gs].unsqueeze(2).broadcast_to((P, t, 2 * NT))
        nc.vector.tensor_tensor(
            idx[:].rearrange("p (g c) -> p g c", c=2 * NT),
            cidx[:, : t * 2 * NT].rearrange("p (g c) -> p g c", c=2 * NT),
            m2_b, OP.add)

        # scatter taps into per-row absolute bins (two calls per superblock)
        dst = pool.tile([P, SUP, W], F32, tag="dst", name="dst")[:, :t]
        s0 = 0
        while s0 < t:
            ts = min(T_SC, t - s0)
            ss = slice(s0, s0 + ts)
            if "scatter" in ablate:
                nc.vector.tensor_copy(dst[:, ss, :NT], q_t[:, ss])
            else:
                nc.gpsimd.local_scatter(
                    dst[:, ss].rearrange("p g w -> p (g w)").bitcast(I16),
                    q_t[:, ss].rearrange("p g n -> p (g n)").bitcast(I16),
                    idx[:, s0 * 2 * NT: (s0 + ts) * 2 * NT],
                    channels=P, num_elems=ts * W * 2, num_idxs=ts * 2 * NT,
                )
            s0 += ts

        # fold edges, copy interior, write out
        out_t = pool.tile([P, SUP, A], F32, tag="out", name="out")[:, :t]
        nc.vector.tensor_reduce(out_t[:, :, 0:1], dst[:, :, : PAD_LO + 1],
                                AX.X, OP.add)
        nc.vector.tensor_reduce(out_t[:, :, 50:51], dst[:, :, PAD_LO + 50:],
                                AX.X, OP.add)
        nc.scalar.copy(out_t[:, :, 1:50], dst[:, :, PAD_LO + 1: PAD_LO + 50])
        nc.sync.dma_start(o_v[:, gs], out_t[:])

        g0 += t

    if repeat > 1:
        loop_cm.__exit__(None, None, None)


def _build_nc(g_total: int = G, repeat: int = 1, ablate: set | None = None):
    nc = bacc.Bacc("TRN2", target_bir_lowering=False, debug=False,
                   num_devices=N_CORES)
    rows = g_total * P
    ins = [
        nc.dram_tensor("reward", [rows, 1], F32, kind="ExternalInput").ap(),
        nc.dram_tensor("probs", [rows, A], F32, kind="ExternalInput").ap(),
        nc.dram_tensor("not_done", [rows, 1], F32, kind="ExternalInput").ap(),
        nc.dram_tensor("neg_tau", [1, SUP * NT], F32, kind="ExternalInput").ap(),
        nc.dram_tensor("cidx", [1, SUP * 2 * NT], I16, kind="ExternalInput").ap(),
    ]
    outs = [nc.dram_tensor("out", [rows, A], F32, kind="ExternalOutput").ap()]
    with tile.TileContext(nc) as tc:
        with ExitStack() as ctx:
            build_kernel(ctx, tc, outs, ins, g_total=g_total, repeat=repeat,
                         ablate=ablate)
    nc.compile()
    return nc


_COMPILED = {}


def kernel(reward: np.ndarray, probs: np.ndarray, not_done: np.ndarray,
           repeat: int = 1, ablate: frozenset = frozenset()) -> np.ndarray:
    reward = np.ascontiguousarray(np.asarray(reward, dtype=np.float32))
    probs = np.ascontiguousarray(np.asarray(probs, dtype=np.float32))
    not_done = np.ascontiguousarray(np.asarray(not_done, dtype=np.float32))
    assert reward.shape == (BS, 1) and probs.shape == (BS, A)

    key = (G, repeat, ablate)
    if key not in _COMPILED:
        _COMPILED[key] = _build_nc(G, repeat=repeat, ablate=set(ablate))
    nc = _COMPILED[key]

    consts = host_constants()
    in_maps = []
    for c in range(N_CORES):
        sl = slice(c * ROWS, (c + 1) * ROWS)
        in_maps.append({
            "reward": reward[sl],
            "probs": probs[sl],
            "not_done": not_done[sl],
            "neg_tau": consts["neg_tau"],
            "cidx": consts["cidx"],
        })
    res = run_bass_kernel_spmd(nc, in_maps, list(range(N_CORES)))
    out = np.concatenate([res.results[c]["out"] for c in range(N_CORES)], axis=0)
    return out


# revision 15
# speedup vs baseline: 1.2868x; 1.2868x over previous
"""Trainium2 Bass kernel for C51 categorical projection (histogram binning).

Algorithm (per row, all f32):
  alpha = (reward + 0.1)/0.4 = 2.5 r + 0.25;  m = floor(alpha);  f = alpha - m
  not_done=0 rows are rewritten as a point mass sum(p)=1 at atom 25 (value 0),
  which is exact because (r+0.1)/0.4 + 0.99*25 = (r+10)/0.4.
  Atom j lands at position m + j + x_j with x_j = f - 0.01 j, |x_j| < 1, so it
  contributes to "taps" tau in {j-1, j, j+1} relative to m:
      q[tau] = p[tau](1-|x_tau|) + p[tau-1] relu(x_{tau-1}) + p[tau+1] relu(-x_{tau+1})
  Each row's 52 taps are scattered to absolute bins (m + tau) with
  nc.gpsimd.local_scatter (per-partition indices; f32 moved exactly as int16
  pairs), then bins <=0 and >=50 are fold-reduced into output bins 0 and 50.

Sharding: pure data parallel, batch split across 8 cores (65536 rows each).
Rows map to (partition p, column g) via row = p*512 + g; superblocks of
SUP=24 row-groups (3072 rows) share one set of wide DVE ops and issue two
local_scatter calls (scatter scratch caps one call at 12 segments).
"""
from contextlib import ExitStack

import numpy as np

import concourse.bacc as bacc
import concourse.bass as bass
import concourse.tile as tile
import concourse.dve_ops as dve_ops
from concourse import mybir
from concourse.bass_utils import run_bass_kernel_spmd
from concourse.dve_spec import Spec, Src0, Src1, One, Zero, lower, maxx
from concourse.dve_uop import DveOpSpec

# problem constants (hardcoded per harness contract)
BS = 524288
A = 51
N_CORES = 8
ROWS = BS // N_CORES            # 65536 rows per core
P = 128                         # SBUF partitions
G = ROWS // P                   # 512 row-groups per partition
NT = 52                         # taps tau = 0..51
PW = 54                         # padded probs row: [0, p0..p50, 0, 0]
PAD_LO = 14
W = PAD_LO + NT + 14            # 80 f32 bins per segment in scatter dst
T_SC = 12                       # segments per scatter call (scratch cap)
SUP = 2 * T_SC                  # segments per superblock

F32 = mybir.dt.float32
I16 = mybir.dt.int16
I32 = mybir.dt.int32
AX = mybir.AxisListType
OP = mybir.AluOpType


def _register_op(name: str, body, reference):
    """Register a custom DVE op with a runtime-computed uops sha."""
    for op in dve_ops.OPS:
        if op.name == name:
            return op
    spec = Spec(body=body, reference=reference)
    row = dve_ops._CUSTOM_DVE_ROW_BASE + len(dve_ops.OPS)
    shas = {}
    for ver in ("v3", "v4"):
        tmp = DveOpSpec(name=name, opcode=row, uops=lower(spec, ver=ver),
                        rd1_en=True)
        shas[ver] = tmp.sha(ver)
    op = dve_ops.DveOp(name, spec, subdim=False, uops_sha=shas)
    dve_ops.OPS.append(op)
    dve_ops.CUSTOM_DVE_SPECS[op.name] = spec
    dve_ops._SUB_OPCODE_FOR_NAME[op.name] = row
    return op


def _ref2(fn):
    def _r(in0, in1, s0, s1, imm2):
        a = in0.astype(np.float32)
        b = np.asarray(in1, dtype=np.float32).reshape(a.shape)
        return fn(a, b).astype(np.float32)
    return _r


TRI_MUL = _register_op(
    "TRI_MUL_ANT", Src0 * (One - maxx(Src1, Zero - Src1)),
    _ref2(lambda a, b: a * (1.0 - np.abs(b))))
G_MUL = _register_op(
    "G_MUL_ANT", Src0 * maxx(Src1, Zero),
    _ref2(lambda a, b: a * np.maximum(b, 0.0)))
H_MUL = _register_op(
    "H_MUL_ANT", Src0 * maxx(Zero - Src1, Zero),
    _ref2(lambda a, b: a * np.maximum(-b, 0.0)))


def _block_sizes(g_total: int) -> list[int]:
    sizes = []
    g = 0
    while g < g_total:
        t = min(SUP, g_total - g)
        sizes.append(t)
        g += t
    return sizes


def host_constants() -> dict[str, np.ndarray]:
    # neg_tau[g, j] = -0.01 * j for j = 0..51, tiled per segment
    neg_tau = (-0.01 * np.arange(NT, dtype=np.float64)).astype(np.float32)
    neg_tau_full = np.tile(neg_tau, SUP)[None, :]
    # cidx[g, c] = 2*W*(g % T_SC) + 2*PAD_LO + c  (int16), per-scatter-call window
    c = np.arange(2 * NT, dtype=np.int64)
    g = np.arange(SUP, dtype=np.int64) % T_SC
    cidx = (2 * W * g[:, None] + 2 * PAD_LO + c[None, :]).astype(np.int16)
    return {"neg_tau": neg_tau_full, "cidx": cidx.reshape(1, -1)}


def build_kernel(ctx: ExitStack, tc: tile.TileContext, outs, ins,
                 g_total: int = G, repeat: int = 1, ablate: set | None = None):
    ablate = ablate or set()
    nc = tc.nc
    reward_d, probs_d, nd_d, neg_tau_d, cidx_d = ins
    out_d = outs[0]

    r_v = reward_d.rearrange("(p g) o -> p (g o)", p=P)      # [128, G]
    n_v = nd_d.rearrange("(p g) o -> p (g o)", p=P)          # [128, G]
    p_v = probs_d.rearrange("(p g) a -> p g a", p=P)         # [128, G, 51]
    o_v = out_d.rearrange("(p g) a -> p g a", p=P)           # [128, G, 51]

    const = ctx.enter_context(tc.tile_pool(name="const", bufs=1))
    pre = ctx.enter_context(tc.tile_pool(name="pre", bufs=1))
    pool = ctx.enter_context(tc.tile_pool(name="blk", bufs=2))

    neg_tau = const.tile([P, SUP * NT], F32, name="neg_tau")
    nc.sync.dma_start(neg_tau[:], neg_tau_d[:].partition_broadcast(P))
    cidx = const.tile([P, SUP * 2 * NT], I16, name="cidx")
    nc.sync.dma_start(cidx[:], cidx_d[:].partition_broadcast(P))

    if repeat > 1:
        loop_cm = tc.For_i(0, repeat, 1)
        loop_cm.__enter__()

    # ---- prepass: per-row scalars [128, G] ----
    rt = pre.tile([P, g_total], F32, name="rt")
    nc.sync.dma_start(rt[:], r_v[:])
    ndt = pre.tile([P, g_total], F32, name="ndt")
    nc.sync.dma_start(ndt[:], n_v[:])

    alpha = pre.tile([P, g_total], F32, name="alpha")
    nc.vector.tensor_scalar(alpha[:], rt[:], 2.5, 0.25, OP.mult, OP.add)
    m0i = pre.tile([P, g_total], I32, name="m0i")
    nc.vector.tensor_copy(m0i[:], alpha[:])                  # round to nearest
    m0f = pre.tile([P, g_total], F32, name="m0f")
    nc.vector.tensor_copy(m0f[:], m0i[:])
    lt = pre.tile([P, g_total], F32, name="lt")
    nc.vector.tensor_tensor(lt[:], alpha[:], m0f[:], OP.is_lt)   # alpha < m0 -> 1.0
    mf = pre.tile([P, g_total], F32, name="mf")
    nc.vector.tensor_tensor(mf[:], m0f[:], lt[:], OP.subtract)   # m = floor(alpha)
    ft = pre.tile([P, g_total], F32, name="ft")
    nc.vector.tensor_tensor(ft[:], alpha[:], mf[:], OP.subtract)  # f in [0,1)
    m2f = pre.tile([P, g_total], F32, name="m2f")
    # clamp m to the representable shift range, then double (byte-pair index)
    nc.vector.tensor_scalar(m2f[:], mf[:], float(-PAD_LO), float(W - PAD_LO - NT),
                            OP.max, OP.min)
    nc.vector.tensor_scalar_mul(m2f[:], m2f[:], 2.0)
    m2 = pre.tile([P, g_total], I16, name="m2")
    nc.vector.tensor_copy(m2[:], m2f[:])
    ndc = pre.tile([P, g_total], F32, name="ndc")
    nc.vector.tensor_scalar(ndc[:], ndt[:], -1.0, 1.0, OP.mult, OP.add)  # 1 - nd

    # ---- per-superblock loop ----
    g0 = 0
    for t in _block_sizes(g_total):
        gs = slice(g0, g0 + t)

        ppad = pool.tile([P, SUP, PW], F32, tag="ppad", name="ppad")[:, :t]
        nc.sync.dma_start(ppad[:, :, 1:52], p_v[:, gs])
        nc.vector.memset(ppad[:, :, 0:1], 0.0)
        nc.vector.memset(ppad[:, :, 52:54], 0.0)

        # p~ = nd * p (in place); then p~[atom 25] += (1 - nd)  (sum(p) == 1)
        nd_b = ndt[:, gs].unsqueeze(2).broadcast_to((P, t, 51))
        nc.vector.tensor_tensor(ppad[:, :, 1:52], ppad[:, :, 1:52], nd_b, OP.mult)
        nc.vector.tensor_tensor(ppad[:, :, 26:27], ppad[:, :, 26:27],
                                ndc[:, gs].unsqueeze(2), OP.add)

        # x[j] = f - 0.01*j over j = 0..51
        x_t = pool.tile([P, SUP, NT], F32, tag="x", name="x")[:, :t]
        f_b = ft[:, gs].unsqueeze(2).broadcast_to((P, t, NT))
        nc.vector.tensor_tensor(x_t[:], neg_tau[:, : t * NT].rearrange(
            "p (g n) -> p g n", n=NT), f_b, OP.add)

        pm = ppad[:, :, 1:53]                                # p~[j], j = 0..51
        q_t = pool.tile([P, SUP, NT], F32, tag="q", name="q")[:, :t]
        if "q" in ablate:
            nc.vector.tensor_copy(q_t[:], pm)
        else:
            nc.vector._custom_dve(TRI_MUL, out=q_t[:], in0=pm, in1=x_t[:])
            gj = pool.tile([P, SUP, NT], F32, tag="gj", name="gj")[:, :t]
            nc.vector._custom_dve(G_MUL, out=gj[:], in0=pm, in1=x_t[:])
            hj = pool.tile([P, SUP, NT], F32, tag="hj", name="hj")[:, :t]
            nc.vector._custom_dve(H_MUL, out=hj[:], in0=pm, in1=x_t[:])
            nc.vector.tensor_tensor(q_t[:, :, 1:52], q_t[:, :, 1:52],
                                    gj[:, :, 0:51], OP.add)
            nc.vector.tensor_tensor(q_t[:, :, 0:51], q_t[:, :, 0:51],
                                    hj[:, :, 1:52], OP.add)

        # indices: idx[g, c] = 2*W*(g % T_SC) + 2*PAD_LO + c + 2*m  (int16)
        idx = pool.tile([P, SUP * 2 * NT], I16, tag="idx", name="idx")[:, : t * 2 * NT]
        m2_b = m2[:, gs].unsqueeze(2).broadcast_to((P, t, 2 * NT))
        nc.vector.tensor_tensor(
            idx[:].rearrange("p (g c) -> p g c", c=2 * NT),
            cidx[:, : t * 2 * NT].rearrange("p (g c) -> p g c", c=2 * NT),
            m2_b, OP.add)

        # scatter taps into per-row absolute bins (two calls per superblock)
        dst = pool.tile([P, SUP, W], F32, tag="dst", name="dst")[:, :t]
        s0 = 0
        while s0 < t:
            ts = min(T_SC, t - s0)
            ss = slice(s0, s0 + ts)
            if "scatter" in ablate:
                nc.vector.tensor_copy(dst[:, ss, :NT], q_t[:, ss])
            else:
                nc.gpsimd.local_scatter(
                    dst[:, ss].rearrange("p g w -> p (g w)").bitcast(I16),
                    q_t[:, ss].rearrange("p g n -> p (g n)").bitcast(I16),
                    idx[:, s0 * 2 * NT: (s0 + ts) * 2 * NT],
                    channels=P, num_elems=ts * W * 2, num_idxs=ts * 2 * NT,
                )
            s0 += ts

        # fold edges, copy interior, write out
        out_t = pool.tile([P, SUP, A], F32, tag="out", name="out")[:, :t]
        nc.vector.tensor_reduce(out_t[:, :, 0:1], dst[:, :, : PAD_LO + 1],
                                AX.X, OP.add)
        nc.vector.tensor_reduce(out_t[:, :, 50:51], dst[:, :, PAD_LO + 50:],
                                AX.X, OP.add)
        nc.vector.tensor_copy(out_t[:, :, 1:50], dst[:, :, PAD_LO + 1: PAD_LO + 50])
        nc.sync.dma_start(o_v[:, gs], out_t[:])

        g0 += t

    if repeat > 1:
        loop_cm.__exit__(None, None, None)


def _build_nc(g_total: int = G, repeat: int = 1, ablate: set | None = None):
    nc = bacc.Bacc("TRN2", target_bir_lowering=False, debug=False,
                   num_devices=N_CORES)
    rows = g_total * P
    ins = [
        nc.dram_tensor("reward", [rows, 1], F32, kind="ExternalInput").ap(),
        nc.dram_tensor("probs", [rows, A], F32, kind="ExternalInput").ap(),
        nc.dram_tensor("not_done", [rows, 1], F32, kind="ExternalInput").ap(),
        nc.dram_tensor("neg_tau", [1, SUP * NT], F32, kind="ExternalInput").ap(),
        nc.dram_tensor("cidx", [1, SUP * 2 * NT], I16, kind="ExternalInput").ap(),
    ]
    outs = [nc.dram_tensor("out", [rows, A], F32, kind="ExternalOutput").ap()]
    with tile.TileContext(nc) as tc:
        with ExitStack() as ctx:
            build_kernel(ctx, tc, outs, ins, g_total=g_total, repeat=repeat,
                         ablate=ablate)
    nc.compile()
    return nc


_COMPILED = {}


def kernel(reward: np.ndarray, probs: np.ndarray, not_done: np.ndarray,
           repeat: int = 1, ablate: frozenset = frozenset()) -> np.ndarray:
    reward = np.ascontiguousarray(np.asarray(reward, dtype=np.float32))
    probs = np.ascontiguousarray(np.asarray(probs, dtype=np.float32))
    not_done = np.ascontiguousarray(np.asarray(not_done, dtype=np.float32))
    assert reward.shape == (BS, 1) and probs.shape == (BS, A)

    key = (G, repeat, ablate)
    if key not in _COMPILED:
        _COMPILED[key] = _build_nc(G, repeat=repeat, ablate=set(ablate))
    nc = _COMPILED[key]

    consts = host_constants()
    in_maps = []
    for c in range(N_CORES):
        sl = slice(c * ROWS, (c + 1) * ROWS)
        in_maps.append({
            "reward": reward[sl],
            "probs": probs[sl],
            "not_done": not_done[sl],
            "neg_tau": consts["neg_tau"],
            "cidx": consts["cidx"],
        })
    res = run_bass_kernel_spmd(nc, in_maps, list(range(N_CORES)))
    out = np.concatenate([res.results[c]["out"] for c in range(N_CORES)], axis=0)
    return out


# revision 16
# speedup vs baseline: 1.5481x; 1.2031x over previous
"""Trainium2 Bass kernel for C51 categorical projection (histogram binning).

Algorithm (per row, all f32):
  alpha = (reward + 0.1)/0.4 = 2.5 r + 0.25;  m = floor(alpha);  f = alpha - m
  not_done=0 rows are rewritten as a point mass sum(p)=1 at atom 25 (value 0),
  which is exact because (r+0.1)/0.4 + 0.99*25 = (r+10)/0.4.
  Atom j lands at position m + j + x_j with x_j = f - 0.01 j, |x_j| < 1, so it
  contributes to "taps" tau in {j-1, j, j+1} relative to m:
      q[tau] = p[tau](1-|x_tau|) + p[tau-1] relu(x_{tau-1}) + p[tau+1] relu(-x_{tau+1})
  Each row's 52 taps are scattered to absolute bins (m + tau) with
  nc.gpsimd.local_scatter (per-partition indices; f32 moved exactly as int16
  pairs), then bins <=0 and >=50 are fold-reduced into output bins 0 and 50.

Sharding: pure data parallel, batch split across 8 cores (65536 rows each).
Rows map to (partition p, column g) via row = p*512 + g; superblocks of
SUP=24 row-groups (3072 rows) share one set of wide DVE ops and issue two
local_scatter calls (scatter scratch caps one call at 12 segments).
"""
from contextlib import ExitStack

import numpy as np

import concourse.bacc as bacc
import concourse.bass as bass
import concourse.tile as tile
import concourse.dve_ops as dve_ops
from concourse import mybir
from concourse.bass_utils import run_bass_kernel_spmd
from concourse.dve_spec import Spec, Src0, Src1, One, Zero, lower, maxx
from concourse.dve_uop import DveOpSpec

# problem constants (hardcoded per harness contract)
BS = 524288
A = 51
N_CORES = 8
ROWS = BS // N_CORES            # 65536 rows per core
P = 128                         # SBUF partitions
G = ROWS // P                   # 512 row-groups per partition
NT = 52                         # taps tau = 0..51
PW = 54                         # padded probs row: [0, p0..p50, 0, 0]
PAD_LO = 14
W = PAD_LO + NT + 14            # 80 f32 bins per segment in scatter dst
T_SC = 12                       # segments per scatter call (scratch cap)
SUP = 2 * T_SC                  # segments per superblock

F32 = mybir.dt.float32
I16 = mybir.dt.int16
I32 = mybir.dt.int32
AX = mybir.AxisListType
OP = mybir.AluOpType


def _register_op(name: str, body, reference):
    """Register a custom DVE op with a runtime-computed uops sha."""
    for op in dve_ops.OPS:
        if op.name == name:
            return op
    spec = Spec(body=body, reference=reference)
    row = dve_ops._CUSTOM_DVE_ROW_BASE + len(dve_ops.OPS)
    shas = {}
    for ver in ("v3", "v4"):
        tmp = DveOpSpec(name=name, opcode=row, uops=lower(spec, ver=ver),
                        rd1_en=True)
        shas[ver] = tmp.sha(ver)
    op = dve_ops.DveOp(name, spec, subdim=False, uops_sha=shas)
    dve_ops.OPS.append(op)
    dve_ops.CUSTOM_DVE_SPECS[op.name] = spec
    dve_ops._SUB_OPCODE_FOR_NAME[op.name] = row
    return op


def _ref2(fn):
    def _r(in0, in1, s0, s1, imm2):
        a = in0.astype(np.float32)
        b = np.asarray(in1, dtype=np.float32).reshape(a.shape)
        return fn(a, b).astype(np.float32)
    return _r


TRI_MUL = _register_op(
    "TRI_MUL_ANT", Src0 * (One - maxx(Src1, Zero - Src1)),
    _ref2(lambda a, b: a * (1.0 - np.abs(b))))
G_MUL = _register_op(
    "G_MUL_ANT", Src0 * maxx(Src1, Zero),
    _ref2(lambda a, b: a * np.maximum(b, 0.0)))
H_MUL = _register_op(
    "H_MUL_ANT", Src0 * maxx(Zero - Src1, Zero),
    _ref2(lambda a, b: a * np.maximum(-b, 0.0)))


def _block_sizes(g_total: int) -> list[int]:
    sizes = []
    g = 0
    while g < g_total:
        t = min(SUP, g_total - g)
        sizes.append(t)
        g += t
    return sizes


def host_constants() -> dict[str, np.ndarray]:
    # neg_tau[g, j] = -0.01 * j for j = 0..51, tiled per segment
    neg_tau = (-0.01 * np.arange(NT, dtype=np.float64)).astype(np.float32)
    neg_tau_full = np.tile(neg_tau, SUP)[None, :]
    # cidx[g, c] = 2*W*(g % T_SC) + 2*PAD_LO + c  (int16), per-scatter-call window
    c = np.arange(2 * NT, dtype=np.int64)
    g = np.arange(SUP, dtype=np.int64) % T_SC
    cidx = (2 * W * g[:, None] + 2 * PAD_LO + c[None, :]).astype(np.int16)
    return {"neg_tau": neg_tau_full, "cidx": cidx.reshape(1, -1)}


def build_kernel(ctx: ExitStack, tc: tile.TileContext, outs, ins,
                 g_total: int = G, repeat: int = 1, ablate: set | None = None):
    ablate = ablate or set()
    nc = tc.nc
    reward_d, probs_d, nd_d, neg_tau_d, cidx_d = ins
    out_d = outs[0]

    r_v = reward_d.rearrange("(p g) o -> p (g o)", p=P)      # [128, G]
    n_v = nd_d.rearrange("(p g) o -> p (g o)", p=P)          # [128, G]
    p_v = probs_d.rearrange("(p g) a -> p g a", p=P)         # [128, G, 51]
    o_v = out_d.rearrange("(p g) a -> p g a", p=P)           # [128, G, 51]

    const = ctx.enter_context(tc.tile_pool(name="const", bufs=1))
    pre = ctx.enter_context(tc.tile_pool(name="pre", bufs=1))
    pool = ctx.enter_context(tc.tile_pool(name="blk", bufs=3))

    neg_tau = const.tile([P, SUP * NT], F32, name="neg_tau")
    nc.sync.dma_start(neg_tau[:], neg_tau_d[:].partition_broadcast(P))
    cidx = const.tile([P, SUP * 2 * NT], I16, name="cidx")
    nc.sync.dma_start(cidx[:], cidx_d[:].partition_broadcast(P))

    if repeat > 1:
        loop_cm = tc.For_i(0, repeat, 1)
        loop_cm.__enter__()

    # ---- prepass: per-row scalars [128, G] ----
    rt = pre.tile([P, g_total], F32, name="rt")
    nc.sync.dma_start(rt[:], r_v[:])
    ndt = pre.tile([P, g_total], F32, name="ndt")
    nc.sync.dma_start(ndt[:], n_v[:])

    alpha = pre.tile([P, g_total], F32, name="alpha")
    nc.vector.tensor_scalar(alpha[:], rt[:], 2.5, 0.25, OP.mult, OP.add)
    m0i = pre.tile([P, g_total], I32, name="m0i")
    nc.vector.tensor_copy(m0i[:], alpha[:])                  # round to nearest
    m0f = pre.tile([P, g_total], F32, name="m0f")
    nc.vector.tensor_copy(m0f[:], m0i[:])
    lt = pre.tile([P, g_total], F32, name="lt")
    nc.vector.tensor_tensor(lt[:], alpha[:], m0f[:], OP.is_lt)   # alpha < m0 -> 1.0
    mf = pre.tile([P, g_total], F32, name="mf")
    nc.vector.tensor_tensor(mf[:], m0f[:], lt[:], OP.subtract)   # m = floor(alpha)
    ft = pre.tile([P, g_total], F32, name="ft")
    nc.vector.tensor_tensor(ft[:], alpha[:], mf[:], OP.subtract)  # f in [0,1)
    m2f = pre.tile([P, g_total], F32, name="m2f")
    # clamp m to the representable shift range, then double (byte-pair index)
    nc.vector.tensor_scalar(m2f[:], mf[:], float(-PAD_LO), float(W - PAD_LO - NT),
                            OP.max, OP.min)
    nc.vector.tensor_scalar_mul(m2f[:], m2f[:], 2.0)
    m2 = pre.tile([P, g_total], I16, name="m2")
    nc.vector.tensor_copy(m2[:], m2f[:])
    ndc = pre.tile([P, g_total], F32, name="ndc")
    nc.vector.tensor_scalar(ndc[:], ndt[:], -1.0, 1.0, OP.mult, OP.add)  # 1 - nd

    # ---- per-superblock loop ----
    g0 = 0
    for t in _block_sizes(g_total):
        gs = slice(g0, g0 + t)

        ppad = pool.tile([P, SUP, PW], F32, tag="ppad", name="ppad")[:, :t]
        nc.sync.dma_start(ppad[:, :, 1:52], p_v[:, gs])
        nc.vector.memset(ppad[:, :, 0:1], 0.0)
        nc.vector.memset(ppad[:, :, 52:54], 0.0)

        # p~ = nd * p (in place); then p~[atom 25] += (1 - nd)  (sum(p) == 1)
        nd_b = ndt[:, gs].unsqueeze(2).broadcast_to((P, t, 51))
        nc.vector.tensor_tensor(ppad[:, :, 1:52], ppad[:, :, 1:52], nd_b, OP.mult)
        nc.vector.tensor_tensor(ppad[:, :, 26:27], ppad[:, :, 26:27],
                                ndc[:, gs].unsqueeze(2), OP.add)

        # x[j] = f - 0.01*j over j = 0..51
        x_t = pool.tile([P, SUP, NT], F32, tag="x", name="x")[:, :t]
        f_b = ft[:, gs].unsqueeze(2).broadcast_to((P, t, NT))
        nc.vector.tensor_tensor(x_t[:], neg_tau[:, : t * NT].rearrange(
            "p (g n) -> p g n", n=NT), f_b, OP.add)

        pm = ppad[:, :, 1:53]                                # p~[j], j = 0..51
        q_t = pool.tile([P, SUP, NT], F32, tag="q", name="q")[:, :t]
        if "q" in ablate:
            nc.vector.tensor_copy(q_t[:], pm)
        else:
            nc.vector._custom_dve(TRI_MUL, out=q_t[:], in0=pm, in1=x_t[:])
            gj = pool.tile([P, SUP, NT], F32, tag="gj", name="gj")[:, :t]
            nc.vector._custom_dve(G_MUL, out=gj[:], in0=pm, in1=x_t[:])
            hj = pool.tile([P, SUP, NT], F32, tag="hj", name="hj")[:, :t]
            nc.vector._custom_dve(H_MUL, out=hj[:], in0=pm, in1=x_t[:])
            nc.vector.tensor_tensor(q_t[:, :, 1:52], q_t[:, :, 1:52],
                                    gj[:, :, 0:51], OP.add)
            nc.vector.tensor_tensor(q_t[:, :, 0:51], q_t[:, :, 0:51],
                                    hj[:, :, 1:52], OP.add)

        # indices: idx[g, c] = 2*W*(g % T_SC) + 2*PAD_LO + c + 2*m  (int16)
        idx = pool.tile([P, SUP * 2 * NT], I16, tag="idx", name="idx")[:, : t * 2 * NT]
        m2_b = m2[:, gs].unsqueeze(2).broadcast_to((P, t, 2 * NT))
        nc.vector.tensor_tensor(
            idx[:].rearrange("p (g c) -> p g c", c=2 * NT),
            cidx[:, : t * 2 * NT].rearrange("p (g c) -> p g c", c=2 * NT),
            m2_b, OP.add)

        # scatter taps into per-row absolute bins (two calls per superblock)
        dst = pool.tile([P, SUP, W], F32, tag="dst", name="dst")[:, :t]
        s0 = 0
        while s0 < t:
            ts = min(T_SC, t - s0)
            ss = slice(s0, s0 + ts)
            if "scatter" in ablate:
                nc.vector.tensor_copy(dst[:, ss, :NT], q_t[:, ss])
            else:
                nc.gpsimd.local_scatter(
                    dst[:, ss].rearrange("p g w -> p (g w)").bitcast(I16),
                    q_t[:, ss].rearrange("p g n -> p (g n)").bitcast(I16),
                    idx[:, s0 * 2 * NT: (s0 + ts) * 2 * NT],
                    channels=P, num_elems=ts * W * 2, num_idxs=ts * 2 * NT,
                )
            s0 += ts

        # fold edges, copy interior, write out
        out_t = pool.tile([P, SUP, A], F32, tag="out", name="out")[:, :t]
        nc.vector.tensor_reduce(out_t[:, :, 0:1], dst[:, :, : PAD_LO + 1],
                                AX.X, OP.add)
        nc.vector.tensor_reduce(out_t[:, :, 50:51], dst[:, :, PAD_LO + 50:],
                                AX.X, OP.add)
        nc.vector.tensor_copy(out_t[:, :, 1:50], dst[:, :, PAD_LO + 1: PAD_LO + 50])
        nc.sync.dma_start(o_v[:, gs], out_t[:])

        g0 += t

    if repeat > 1:
        loop_cm.__exit__(None, None, None)


def _build_nc(g_total: int = G, repeat: int = 1, ablate: set | None = None):
    nc = bacc.Bacc("TRN2", target_bir_lowering=False, debug=False,
                   num_devices=N_CORES)
    rows = g_total * P
    ins = [
        nc.dram_tensor("reward", [rows, 1], F32, kind="ExternalInput").ap(),
        nc.dram_tensor("probs", [rows, A], F32, kind="ExternalInput").ap(),
        nc.dram_tensor("not_done", [rows, 1], F32, kind="ExternalInput").ap(),
        nc.dram_tensor("neg_tau", [1, SUP * NT], F32, kind="ExternalInput").ap(),
        nc.dram_tensor("cidx", [1, SUP * 2 * NT], I16, kind="ExternalInput").ap(),
    ]
    outs = [nc.dram_tensor("out", [rows, A], F32, kind="ExternalOutput").ap()]
    with tile.TileContext(nc) as tc:
        with ExitStack() as ctx:
            build_kernel(ctx, tc, outs, ins, g_total=g_total, repeat=repeat,
                         ablate=ablate)
    nc.compile()
    return nc


_COMPILED = {}


def kernel(reward: np.ndarray, probs: np.ndarray, not_done: np.ndarray,
           repeat: int = 1, ablate: frozenset = frozenset()) -> np.ndarray:
    reward = np.ascontiguousarray(np.asarray(reward, dtype=np.float32))
    probs = np.ascontiguousarray(np.asarray(probs, dtype=np.float32))
    not_done = np.ascontiguousarray(np.asarray(not_done, dtype=np.float32))
    assert reward.shape == (BS, 1) and probs.shape == (BS, A)

    key = (G, repeat, ablate)
    if key not in _COMPILED:
        _COMPILED[key] = _build_nc(G, repeat=repeat, ablate=set(ablate))
    nc = _COMPILED[key]

    consts = host_constants()
    in_maps = []
    for c in range(N_CORES):
        sl = slice(c * ROWS, (c + 1) * ROWS)
        in_maps.append({
            "reward": reward[sl],
            "probs": probs[sl],
            "not_done": not_done[sl],
            "neg_tau": consts["neg_tau"],
            "cidx": consts["cidx"],
        })
    res = run_bass_kernel_spmd(nc, in_maps, list(range(N_CORES)))
    out = np.concatenate([res.results[c]["out"] for c in range(N_CORES)], axis=0)
    return out
